# revision 12
# baseline (speedup 1.0000x reference)
"""Trainium2 Bass kernel: y = LP(square(BP(x))) cascaded-biquad IIR filtering.

x: [16, 16384, 64] fp32; bp_sos/lp_sos: [2, 6] second-order sections.
Reference applies, per (batch, channel) sequence along time:
  w = sosfilt(w, bp_sos); w = sosfilt(w*w, lp_sos)
with zero initial conditions (Direct Form I biquads).

Strategy (overlap-save FIR, no recurrence on device):
  Each 2-biquad cascade is an order-4 IIR whose impulse response h decays
  below 1e-12 within 256 samples (pole radii <= 0.84).  So the filter is,
  to fp16 precision, an FIR of 256 taps.  Chunking time into L=128 blocks
  (layout: partition = time-within-chunk, free = chunk*seq), each output
  chunk is EXACTLY two PE matmuls accumulated in PSUM:
     Y_c = A^T @ X_{c-1} + B^T @ X_c,   A[tau,t] = h[128+t-tau],
                                        B[tau,t] = h[t-tau] (t>=tau)
  i.e. pure feed-forward: no chunk-boundary state, no tail gathers, no
  sequential dependencies.  The first chunk reads a zeroed pad block
  (exact: zero initial conditions).

Per core: 128 sequences (rows of B*C=1024 split 8 ways), everything fp16
on device (inputs, weights, intermediates, output) with fp32 PSUM
accumulation; measured end-to-end error vs the float64 recurrence ~9e-4.

Engine budget per core (@ 2.4GHz PE, 1.2GHz Act, 0.96GHz DVE):
  PE:  2 filters x 2 passes x 16384 cols   = 65536 cy = 27.3 us
  Act: 16 squares  [128,1024] PSUM->SBUF   ~ 16 us
  DVE: 16 copies   [128,1024] PSUM->SBUF   ~ 19 us
  DMA: 4 MiB in + 4 MiB out @ ~360 GB/s    ~ 23 us

Timeline notes (perfetto, exec window = [first useful inst, last inst]):
  walrus preamble ends ~5.9us (free: before first useful inst = the
  framework const memsets); walrus FINI (51 sem clears on PE, ~6.1us +
  barriers) is serialized after the last store's DMA receipt and counts.
  So the shape is: lead-in ~1us | warmup+cold mms | 128 N=512 matmuls
  ~28us | last-pair drain chain | fini ~6.7us.
"""

import numpy as np

# ---------------------------------------------------------------- constants
B, T, C = 16, 16384, 64
NCORES = 8
L = 128           # chunk length == PE contraction depth
N = T // L        # 128 chunks per sequence
S = 128           # sequences per core
TILE = 512        # matmul moving free-dim (one PSUM bank of fp32)
PAIR = 2 * TILE   # consumer granularity (2 PSUM banks)
NP_ = (N * S) // PAIR  # 16 pairs per filter
NWARM = 28        # warmup matmuls: PE busy ~7.5us -> ~10.9us.  The HAM
                  # clock-gate needs ~3.4us of CONTINUOUS PE busy to raise
                  # the PE clock 1.2->2.4GHz; any idle gap while warming
                  # resets the window (measured: a 0.7us gap cost ~2.4us of
                  # half-rate matmuls).  Early input DMA crawls at ~160B/ns,
                  # so pieces 0+1 are only resident ~10.8us -- warm up until
                  # then and enter the real stream without a gap.


def _combine_sos(sos):
    """[2,6] sos -> normalized order-4 (b[0..4], a[0..4]) float64, a[0]=1."""
    sos = np.asarray(sos, dtype=np.float64)
    b1, a1 = sos[0, :3] / sos[0, 3], sos[0, 3:] / sos[0, 3]
    b2, a2 = sos[1, :3] / sos[1, 3], sos[1, 3:] / sos[1, 3]
    return np.convolve(b1, b2), np.convolve(a1, a2)


def _impulse(b, a, n):
    """First n samples of the impulse response of the order-4 IIR (b, a)."""
    u = np.zeros(n + 4)
    y = np.zeros(n + 4)
    u[4] = 1.0
    h = np.zeros(n)
    for t in range(n):
        acc = b[0] * u[t + 4] + b[1] * u[t + 3] + b[2] * u[t + 2] \
            + b[3] * u[t + 1] + b[4] * u[t]
        acc -= a[1] * y[t + 3] + a[2] * y[t + 2] + a[3] * y[t + 1] + a[4] * y[t]
        y[t + 4] = acc
        h[t] = acc
    return h


def _fir_weights(sos):
    """(lhsT_list, n_passes): lhsT_j[tau, t] = h[j*L + t - tau] fp16.

    Pass j multiplies chunk c-j's inputs; j=0 is lower-triangular (causal).
    n_passes chosen so the discarded tail of h is < 1e-5 of peak.
    """
    bb, aa = _combine_sos(sos)
    h = _impulse(bb, aa, 6 * L)
    scale = np.abs(h).max()
    P = 2
    while P < 5 and np.abs(h[P * L:]).max() > 1e-5 * scale:
        P += 1
    if np.abs(h[P * L:]).max() > 1e-5 * scale:
        raise ValueError("impulse response does not decay within 5 chunks")
    idx = np.arange(L)
    K = idx[None, :] - idx[:, None]          # t - tau in [-127, 127]
    mats = []
    for j in range(P - 1, -1, -1):           # oldest history first
        M = h[np.clip(j * L + K, 0, 6 * L - 1)]
        if j == 0:
            M = np.where(K >= 0, M, 0.0)
        mats.append(np.ascontiguousarray(M.astype(np.float16)))
    return mats, P


# ---------------------------------------------------------------- device IR
_PROGRAM_CACHE = {}


def _build_program(p1, p2):
    """p1/p2: number of FIR passes for filter 1 / filter 2 (usually 2)."""
    import concourse.bass as bass
    import concourse.mybir as mybir
    import concourse.tile as tile
    from concourse import bacc

    F32 = mybir.dt.float32
    F16 = mybir.dt.float16
    ts = bass.ts
    PAD1 = (p1 - 1) * S   # zero-pad cols in front of filter-1 input
    PAD2 = (p2 - 1) * S

    class FastExitTileContext(tile.TileContext):
        """TileContext whose exit emits NO drain/barrier/clears at all.

        The walrus codegen FINI that follows the kernel body already
        performs per-engine DRAINs (covering outstanding HWDGE DMA) and two
        all-engine barriers around its own semaphore-clear block, so the
        tile-level drain+barrier+clear storm is redundant for a one-shot
        kernel: Bass.__init__ emits a full-range gpsimd semaphore clear in
        the program PROLOGUE, so the next execution starts clean
        regardless of the state this one leaves behind.
        """

        def _drain_and_barrier(self, tick_clock, wait_clock):
            popped = self.nc._tile_sem_poison_stack.pop()
            assert popped is self._sem_poison
            assert self.sems is not None
            # Free the handles for bookkeeping but emit no instructions.
            for handle in self.sems.allocated().values():
                self.nc.release_semaphore(handle)

    nc = bacc.Bacc(None)
    x_d = nc.declare_dram_parameter("x", [128, T], F16, isOutput=False)
    # both passes of each filter's weights in ONE tensor -> one DMA each
    w1_d = nc.declare_dram_parameter("w1", [128, 128 * p1], F16, False)
    w2_d = nc.declare_dram_parameter("w2", [128, 128 * p2], F16, False)
    out_d = nc.declare_dram_parameter("out", [128, T], F16, isOutput=True)

    with FastExitTileContext(nc) as tc:
        with (
            tc.tile_pool(name="big", bufs=1) as bigpool,
            tc.tile_pool(name="consts", bufs=1) as cpool,
            tc.tile_pool(name="ps", bufs=4, space=bass.MemorySpace.PSUM) as pspool,
        ):
            xbuf = bigpool.tile([128, PAD1 + T], F16, tag="xbuf", name="xbuf")
            bufB = bigpool.tile([128, PAD2 + T], F16, tag="bufB", name="bufB")
            outsb = bigpool.tile([128, T], F16, tag="outsb", name="outsb")
            w1c = cpool.tile([128, 128 * p1], F16, tag="w1c", name="w1c")
            w2c = cpool.tile([128, 128 * p2], F16, tag="w2c", name="w2c")
            w1 = [w1c[:, 128 * j: 128 * (j + 1)] for j in range(p1)]
            w2 = [w2c[:, 128 * j: 128 * (j + 1)] for j in range(p2)]

            # zero pads (exact zero initial conditions)
            nc.vector.memzero(xbuf[:, 0:PAD1])
            nc.vector.memzero(bufB[:, 0:PAD2])

            def sdma(q, out_ap, in_ap):
                w = out_ap.shape[-1]
                k = 1
                for cand in (16, 8, 4, 2):
                    if w % cand == 0 and (w * 2) // cand >= 1024:
                        k = cand
                        break
                if k > 1:
                    out_ap = out_ap.rearrange("p (k w) -> p k w", k=k)
                    in_ap = in_ap.rearrange("p (k w) -> p k w", k=k)
                q.dma_start(out=out_ap, in_=in_ap)

            # Weights on the scalar HWDGE ring (w1c lands ~9.4us), input on
            # the sync ring with a small 512-col lead piece (lands ~9.2us);
            # both rings' transfers interleave on the 16 SDMA engines.
            # (SWDGE/gpsimd was tried for the lead transfers and is far
            # worse: ~2.5-3.5us issue-to-complete, serialized on Q7.)
            sdma(nc.scalar, w1c[:], w1_d[:])
            # piece 1 rides the scalar ring so pieces 0+1 land in parallel
            sdma(nc.scalar, xbuf[:, PAD1 + 512: PAD1 + 1536], x_d[:, 512:1536])
            sdma(nc.scalar, w2c[:], w2_d[:])
            sdma(nc.sync, xbuf[:, PAD1: PAD1 + 512], x_d[:, 0:512])
            for a in range(1536, T, 1024):
                b = min(a + 1024, T)
                sdma(nc.sync, xbuf[:, PAD1 + a: PAD1 + b], x_d[:, a:b])

            # Warmup matmuls on the zeroed pad keep the PE busy (and the HAM
            # clock-gate window filling) from the end of the framework
            # preamble (~7.0us) until w1 + the first input piece land.
            wps = pspool.tile([128, PAIR], F32, tag="ps", name="ps_warm")
            for _ in range(NWARM):
                nc.tensor.matmul(wps[:, 0:128], xbuf[:, 0:128],
                                 xbuf[:, 0:128], start=True, stop=True,
                                 skip_group_check=True)

            def emit_pair(IN, ws, p, consumer, bank_major=False):
                """P passes x 2 matmuls (one per PSUM bank; >512 fp32 out cols
                per matmul fails the ISA check) -> one consumer op.

                bank_major=True orders (bankA j0, j1), (bankB j0, j1) so the
                first pair's input-column needs form a staircase (384, 512,
                896, 1024) instead of needing col 896 by the second matmul.
                """
                c0 = p * PAIR
                ps = pspool.tile([128, PAIR], F32, tag="ps", name="ps")
                nj = len(ws)
                if bank_major:
                    for b in range(2):
                        for j in range(nj):
                            off = c0 + j * S + b * TILE
                            nc.tensor.matmul(
                                ps[:, b * TILE:(b + 1) * TILE], ws[j][:],
                                IN[:, off: off + TILE],
                                start=(j == 0), stop=(j == nj - 1),
                                skip_group_check=True)
                else:
                    for j in range(nj):
                        off = c0 + j * S
                        first, last = (j == 0), (j == nj - 1)
                        nc.tensor.matmul(ps[:, 0:TILE], ws[j][:],
                                         IN[:, off: off + TILE],
                                         start=first, stop=last,
                                         skip_group_check=True)
                        nc.tensor.matmul(ps[:, TILE:PAIR], ws[j][:],
                                         IN[:, off + TILE: off + PAIR],
                                         start=first, stop=last,
                                         skip_group_check=True)
                consumer(p, ps)

            # Filters interleaved on the PE at pair granularity with a small
            # lag.  Consumers: Act (activation square, ~1.0us/pair) drains
            # filter 1, DVE (cast copy, ~1.2us/pair) drains filter 2 -- each
            # engine sees one pair per TWO PE pair-slots (~1.7us), so PSUM
            # never backpressures the matmuls.
            def f1_consumer(p, ps):
                nc.scalar.square(
                    bufB[:, PAD2 + p * PAIR: PAD2 + (p + 1) * PAIR], ps[:])

            def f1_consumer_split(p, ps):
                # final f1 pair: two Act squares, split 640/384 -- the first
                # covers exactly the columns the last f2 unit A reads
                # (c0..c0+640), so its matmuls start ~0.7us after f1's
                # instead of ~1.0us.  (DVE can't square from PSUM: only one
                # PSUM read port, tensor_tensor needs two reads.)
                c0 = PAD2 + p * PAIR
                cut = TILE + S
                nc.scalar.square(bufB[:, c0: c0 + cut], ps[:, 0:cut])
                nc.scalar.square(bufB[:, c0 + cut: c0 + PAIR],
                                 ps[:, cut:PAIR])

            def f2_consumer(p, ps, eng):
                if eng == "split":
                    # penultimate pair: drain both PSUM banks concurrently
                    # on Act + DVE, ONE whole-pair store.
                    c0 = p * PAIR
                    nc.scalar.copy(outsb[:, c0: c0 + TILE], ps[:, 0:TILE])
                    nc.vector.tensor_copy(outsb[:, c0 + TILE: c0 + PAIR],
                                          ps[:, TILE:PAIR])
                    sdma(nc.sync, out_d[:, ts(p, PAIR)], outsb[:, ts(p, PAIR)])
                    return
                dst = outsb[:, ts(p, PAIR)]
                if eng == "v":
                    nc.vector.tensor_copy(dst, ps[:])
                else:
                    nc.scalar.copy(dst, ps[:])
                sdma(nc.sync, out_d[:, ts(p, PAIR)], outsb[:, ts(p, PAIR)])

            def emit_f2_final_units(p):
                """Last f2 pair as two 512-col units so the tail chain
                (consumer -> store issue -> DMA receipt) is per-512."""
                c0 = p * PAIR
                for u in range(2):
                    ps = pspool.tile([128, PAIR], F32, tag="ps",
                                     name="ps_final")
                    for j in range(p2):
                        off = c0 + j * S + u * TILE
                        nc.tensor.matmul(ps[:, 0:TILE], w2[j][:],
                                         bufB[:, off: off + TILE],
                                         start=(j == 0), stop=(j == p2 - 1),
                                         skip_group_check=True)
                    # both unit copies on Act (DVE pays a long post-op DRAIN
                    # after its pair-14 half; Act is free here); stores on
                    # separate queues so the two ~0.6us issues overlap.
                    dst = outsb[:, c0 + u * TILE: c0 + (u + 1) * TILE]
                    nc.scalar.copy(dst, ps[:, 0:TILE])
                    q = nc.sync if u == 0 else nc.scalar
                    sdma(q, out_d[:, c0 + u * TILE: c0 + (u + 1) * TILE], dst)

            LAG = 2
            for p in range(NP_):
                cons = f1_consumer_split if p == NP_ - 1 else f1_consumer
                emit_pair(xbuf, w1, p, cons, bank_major=(p == 0))
                if p >= LAG:
                    emit_pair(bufB, w2, p - LAG,
                              lambda q, ps: f2_consumer(q, ps, "v"))
            emit_pair(bufB, w2, NP_ - 2,
                      lambda q, ps: f2_consumer(q, ps, "split"))
            emit_f2_final_units(NP_ - 1)

    nc.compile()
    return nc


def _get_program(p1, p2):
    key = (p1, p2)
    if key not in _PROGRAM_CACHE:
        _PROGRAM_CACHE[key] = _build_program(p1, p2)
    return _PROGRAM_CACHE[key]


# ---------------------------------------------------------------- host entry
def _shard_inputs(x):
    """x [B,T,C] fp32 -> list of per-core Xm [128, T] fp16 arrays.

    Xm[l, c*S + s] = seq[core*S + s][c*L + l].
    """
    xs = np.ascontiguousarray(np.transpose(np.asarray(x, dtype=np.float32),
                                           (0, 2, 1))).reshape(B * C, T)
    xs = xs.astype(np.float16)
    shards = []
    for core in range(NCORES):
        seqs = xs[core * S: (core + 1) * S]
        Xm = np.ascontiguousarray(
            seqs.reshape(S, N, L).transpose(2, 1, 0)).reshape(L, N * S)
        shards.append(Xm)
    return shards


def _unshard_output(outs):
    """list of per-core [128, T] fp16 device outputs -> [B, T, C] fp32."""
    ys = np.empty((B * C, T), dtype=np.float32)
    for core in range(NCORES):
        O = np.asarray(outs[core]).astype(np.float32)
        ys[core * S: (core + 1) * S] = (
            O.reshape(L, N, S).transpose(2, 1, 0).reshape(S, T))
    return np.ascontiguousarray(ys.reshape(B, C, T).transpose(0, 2, 1))


def kernel(x, bp_sos, lp_sos, _trace=False, **_ignored):
    from concourse.bass_utils import run_bass_kernel_spmd

    w1, p1 = _fir_weights(np.asarray(bp_sos))
    w2, p2 = _fir_weights(np.asarray(lp_sos))
    consts = {
        "w1": np.ascontiguousarray(np.concatenate(w1, axis=1)),
        "w2": np.ascontiguousarray(np.concatenate(w2, axis=1)),
    }
    shards = _shard_inputs(x)
    nc = _get_program(p1, p2)
    in_maps = [dict(consts, x=shards[core]) for core in range(NCORES)]
    res = run_bass_kernel_spmd(nc, in_maps, list(range(NCORES)), trace=_trace)
    out = _unshard_output([res.results[core]["out"] for core in range(NCORES)])
    if _trace:
        return out, res
    return out


if __name__ == "__main__":
    rng = np.random.default_rng(0)
    x = rng.standard_normal((B, T, C), dtype=np.float32)
    sh = _shard_inputs(x)
    rt = _unshard_output(sh)
    print("roundtrip max err (fp16 quant only):",
          np.abs(rt - x).max())


# revision 13
# speedup vs baseline: 1.0460x; 1.0460x over previous
"""Trainium2 Bass kernel: y = LP(square(BP(x))) cascaded-biquad IIR filtering.

x: [16, 16384, 64] fp32; bp_sos/lp_sos: [2, 6] second-order sections.
Reference applies, per (batch, channel) sequence along time:
  w = sosfilt(w, bp_sos); w = sosfilt(w*w, lp_sos)
with zero initial conditions (Direct Form I biquads).

Strategy (overlap-save FIR, no recurrence on device):
  Each 2-biquad cascade is an order-4 IIR whose impulse response h decays
  below 1e-12 within 256 samples (pole radii <= 0.84).  So the filter is,
  to fp16 precision, an FIR of 256 taps.  Chunking time into L=128 blocks
  (layout: partition = time-within-chunk, free = chunk*seq), each output
  chunk is EXACTLY two PE matmuls accumulated in PSUM:
     Y_c = A^T @ X_{c-1} + B^T @ X_c,   A[tau,t] = h[128+t-tau],
                                        B[tau,t] = h[t-tau] (t>=tau)
  i.e. pure feed-forward: no chunk-boundary state, no tail gathers, no
  sequential dependencies.  The first chunk reads a zeroed pad block
  (exact: zero initial conditions).

Per core: 128 sequences (rows of B*C=1024 split 8 ways), everything fp16
on device (inputs, weights, intermediates, output) with fp32 PSUM
accumulation; measured end-to-end error vs the float64 recurrence ~9e-4.

Engine budget per core (@ 2.4GHz PE, 1.2GHz Act, 0.96GHz DVE):
  PE:  2 filters x 2 passes x 16384 cols   = 65536 cy = 27.3 us
  Act: 16 squares  [128,1024] PSUM->SBUF   ~ 16 us
  DVE: 16 copies   [128,1024] PSUM->SBUF   ~ 19 us
  DMA: 4 MiB in + 4 MiB out @ ~360 GB/s    ~ 23 us

Timeline notes (perfetto, exec window = [first useful inst, last inst]):
  walrus preamble ends ~5.9us (free: before first useful inst = the
  framework const memsets); walrus FINI (51 sem clears on PE, ~6.1us +
  barriers) is serialized after the last store's DMA receipt and counts.
  So the shape is: lead-in ~1us | warmup+cold mms | 128 N=512 matmuls
  ~28us | last-pair drain chain | fini ~6.7us.
"""

import numpy as np

# ---------------------------------------------------------------- constants
B, T, C = 16, 16384, 64
NCORES = 8
L = 128           # chunk length == PE contraction depth
N = T // L        # 128 chunks per sequence
S = 128           # sequences per core
TILE = 512        # matmul moving free-dim (one PSUM bank of fp32)
PAIR = 2 * TILE   # consumer granularity (2 PSUM banks)
NP_ = (N * S) // PAIR  # 16 pairs per filter
NWARM = 28        # warmup matmuls: PE busy ~7.5us -> ~10.9us.  The HAM
                  # clock-gate needs ~3.4us of CONTINUOUS PE busy to raise
                  # the PE clock 1.2->2.4GHz; any idle gap while warming
                  # resets the window (measured: a 0.7us gap cost ~2.4us of
                  # half-rate matmuls).  Early input DMA crawls at ~160B/ns,
                  # so pieces 0+1 are only resident ~10.8us -- warm up until
                  # then and enter the real stream without a gap.


def _combine_sos(sos):
    """[2,6] sos -> normalized order-4 (b[0..4], a[0..4]) float64, a[0]=1."""
    sos = np.asarray(sos, dtype=np.float64)
    b1, a1 = sos[0, :3] / sos[0, 3], sos[0, 3:] / sos[0, 3]
    b2, a2 = sos[1, :3] / sos[1, 3], sos[1, 3:] / sos[1, 3]
    return np.convolve(b1, b2), np.convolve(a1, a2)


def _impulse(b, a, n):
    """First n samples of the impulse response of the order-4 IIR (b, a)."""
    u = np.zeros(n + 4)
    y = np.zeros(n + 4)
    u[4] = 1.0
    h = np.zeros(n)
    for t in range(n):
        acc = b[0] * u[t + 4] + b[1] * u[t + 3] + b[2] * u[t + 2] \
            + b[3] * u[t + 1] + b[4] * u[t]
        acc -= a[1] * y[t + 3] + a[2] * y[t + 2] + a[3] * y[t + 1] + a[4] * y[t]
        y[t + 4] = acc
        h[t] = acc
    return h


def _fir_weights(sos):
    """(lhsT_list, n_passes): lhsT_j[tau, t] = h[j*L + t - tau] fp16.

    Pass j multiplies chunk c-j's inputs; j=0 is lower-triangular (causal).
    n_passes chosen so the discarded tail of h is < 1e-5 of peak.
    """
    bb, aa = _combine_sos(sos)
    h = _impulse(bb, aa, 6 * L)
    scale = np.abs(h).max()
    P = 2
    while P < 5 and np.abs(h[P * L:]).max() > 1e-5 * scale:
        P += 1
    if np.abs(h[P * L:]).max() > 1e-5 * scale:
        raise ValueError("impulse response does not decay within 5 chunks")
    idx = np.arange(L)
    K = idx[None, :] - idx[:, None]          # t - tau in [-127, 127]
    mats = []
    for j in range(P - 1, -1, -1):           # oldest history first
        M = h[np.clip(j * L + K, 0, 6 * L - 1)]
        if j == 0:
            M = np.where(K >= 0, M, 0.0)
        mats.append(np.ascontiguousarray(M.astype(np.float16)))
    return mats, P


# ---------------------------------------------------------------- device IR
_PROGRAM_CACHE = {}


def _build_program(p1, p2):
    """p1/p2: number of FIR passes for filter 1 / filter 2 (usually 2)."""
    import concourse.bass as bass
    import concourse.mybir as mybir
    import concourse.tile as tile
    from concourse import bacc

    F32 = mybir.dt.float32
    F16 = mybir.dt.float16
    ts = bass.ts
    PAD1 = (p1 - 1) * S   # zero-pad cols in front of filter-1 input
    PAD2 = (p2 - 1) * S

    class FastExitTileContext(tile.TileContext):
        """TileContext whose exit emits NO drain/barrier/clears at all.

        The walrus codegen FINI that follows the kernel body already
        performs per-engine DRAINs (covering outstanding HWDGE DMA) and two
        all-engine barriers around its own semaphore-clear block, so the
        tile-level drain+barrier+clear storm is redundant for a one-shot
        kernel: Bass.__init__ emits a full-range gpsimd semaphore clear in
        the program PROLOGUE, so the next execution starts clean
        regardless of the state this one leaves behind.
        """

        def _drain_and_barrier(self, tick_clock, wait_clock):
            popped = self.nc._tile_sem_poison_stack.pop()
            assert popped is self._sem_poison
            assert self.sems is not None
            # Free the handles for bookkeeping but emit no instructions.
            for handle in self.sems.allocated().values():
                self.nc.release_semaphore(handle)

    nc = bacc.Bacc(None)
    x_d = nc.declare_dram_parameter("x", [128, T], F16, isOutput=False)
    # both passes of each filter's weights in ONE tensor -> one DMA each
    w1_d = nc.declare_dram_parameter("w1", [128, 128 * p1], F16, False)
    w2_d = nc.declare_dram_parameter("w2", [128, 128 * p2], F16, False)
    out_d = nc.declare_dram_parameter("out", [128, T], F16, isOutput=True)

    with FastExitTileContext(nc) as tc:
        with (
            tc.tile_pool(name="big", bufs=1) as bigpool,
            tc.tile_pool(name="consts", bufs=1) as cpool,
            tc.tile_pool(name="ps", bufs=4, space=bass.MemorySpace.PSUM) as pspool,
        ):
            xbuf = bigpool.tile([128, PAD1 + T], F16, tag="xbuf", name="xbuf")
            bufB = bigpool.tile([128, PAD2 + T], F16, tag="bufB", name="bufB")
            outsb = bigpool.tile([128, T], F16, tag="outsb", name="outsb")
            w1c = cpool.tile([128, 128 * p1], F16, tag="w1c", name="w1c")
            w2c = cpool.tile([128, 128 * p2], F16, tag="w2c", name="w2c")
            w1 = [w1c[:, 128 * j: 128 * (j + 1)] for j in range(p1)]
            w2 = [w2c[:, 128 * j: 128 * (j + 1)] for j in range(p2)]

            # zero pads (exact zero initial conditions)
            nc.vector.memzero(xbuf[:, 0:PAD1])
            nc.vector.memzero(bufB[:, 0:PAD2])

            def sdma(q, out_ap, in_ap):
                w = out_ap.shape[-1]
                k = 1
                for cand in (16, 8, 4, 2):
                    if w % cand == 0 and (w * 2) // cand >= 1024:
                        k = cand
                        break
                if k > 1:
                    out_ap = out_ap.rearrange("p (k w) -> p k w", k=k)
                    in_ap = in_ap.rearrange("p (k w) -> p k w", k=k)
                q.dma_start(out=out_ap, in_=in_ap)

            # Weights on the scalar HWDGE ring (w1c lands ~9.4us), input on
            # the sync ring with a small 512-col lead piece (lands ~9.2us);
            # both rings' transfers interleave on the 16 SDMA engines.
            # (SWDGE/gpsimd was tried for the lead transfers and is far
            # worse: ~2.5-3.5us issue-to-complete, serialized on Q7.)
            sdma(nc.scalar, w1c[:], w1_d[:])
            sdma(nc.scalar, w2c[:], w2_d[:])
            sdma(nc.sync, xbuf[:, PAD1: PAD1 + 512], x_d[:, 0:512])
            for a in range(512, T, 1024):
                b = min(a + 1024, T)
                sdma(nc.sync, xbuf[:, PAD1 + a: PAD1 + b], x_d[:, a:b])

            # Warmup matmuls on the zeroed pad keep the PE busy (and the HAM
            # clock-gate window filling) from the end of the framework
            # preamble (~7.0us) until w1 + the first input piece land.
            wps = pspool.tile([128, PAIR], F32, tag="ps", name="ps_warm")
            for _ in range(NWARM):
                nc.tensor.matmul(wps[:, 0:128], xbuf[:, 0:128],
                                 xbuf[:, 0:128], start=True, stop=True,
                                 skip_group_check=True)

            def emit_pair(IN, ws, p, consumer, bank_major=False):
                """P passes x 2 matmuls (one per PSUM bank; >512 fp32 out cols
                per matmul fails the ISA check) -> one consumer op.

                bank_major=True orders (bankA j0, j1), (bankB j0, j1) so the
                first pair's input-column needs form a staircase (384, 512,
                896, 1024) instead of needing col 896 by the second matmul.
                """
                c0 = p * PAIR
                ps = pspool.tile([128, PAIR], F32, tag="ps", name="ps")
                nj = len(ws)
                if bank_major:
                    for b in range(2):
                        for j in range(nj):
                            off = c0 + j * S + b * TILE
                            nc.tensor.matmul(
                                ps[:, b * TILE:(b + 1) * TILE], ws[j][:],
                                IN[:, off: off + TILE],
                                start=(j == 0), stop=(j == nj - 1),
                                skip_group_check=True)
                else:
                    for j in range(nj):
                        off = c0 + j * S
                        first, last = (j == 0), (j == nj - 1)
                        nc.tensor.matmul(ps[:, 0:TILE], ws[j][:],
                                         IN[:, off: off + TILE],
                                         start=first, stop=last,
                                         skip_group_check=True)
                        nc.tensor.matmul(ps[:, TILE:PAIR], ws[j][:],
                                         IN[:, off + TILE: off + PAIR],
                                         start=first, stop=last,
                                         skip_group_check=True)
                consumer(p, ps)

            # Filters interleaved on the PE at pair granularity with a small
            # lag.  Consumers: Act (activation square, ~1.0us/pair) drains
            # filter 1, DVE (cast copy, ~1.2us/pair) drains filter 2 -- each
            # engine sees one pair per TWO PE pair-slots (~1.7us), so PSUM
            # never backpressures the matmuls.
            def f1_consumer(p, ps):
                nc.scalar.square(
                    bufB[:, PAD2 + p * PAIR: PAD2 + (p + 1) * PAIR], ps[:])

            def f1_consumer_split(p, ps):
                # final f1 pair: two Act squares, split 640/384 -- the first
                # covers exactly the columns the last f2 unit A reads
                # (c0..c0+640), so its matmuls start ~0.7us after f1's
                # instead of ~1.0us.  (DVE can't square from PSUM: only one
                # PSUM read port, tensor_tensor needs two reads.)
                c0 = PAD2 + p * PAIR
                cut = TILE + S
                nc.scalar.square(bufB[:, c0: c0 + cut], ps[:, 0:cut])
                nc.scalar.square(bufB[:, c0 + cut: c0 + PAIR],
                                 ps[:, cut:PAIR])

            def f2_consumer(p, ps, eng):
                if eng == "split":
                    # penultimate pair: drain both PSUM banks concurrently
                    # on Act + DVE, ONE whole-pair store.
                    c0 = p * PAIR
                    nc.scalar.copy(outsb[:, c0: c0 + TILE], ps[:, 0:TILE])
                    nc.vector.tensor_copy(outsb[:, c0 + TILE: c0 + PAIR],
                                          ps[:, TILE:PAIR])
                    sdma(nc.sync, out_d[:, ts(p, PAIR)], outsb[:, ts(p, PAIR)])
                    return
                dst = outsb[:, ts(p, PAIR)]
                if eng == "v":
                    nc.vector.tensor_copy(dst, ps[:])
                else:
                    nc.scalar.copy(dst, ps[:])
                sdma(nc.sync, out_d[:, ts(p, PAIR)], outsb[:, ts(p, PAIR)])

            def emit_f2_final_units(p):
                """Last f2 pair as two 512-col units so the tail chain
                (consumer -> store issue -> DMA receipt) is per-512."""
                c0 = p * PAIR
                for u in range(2):
                    ps = pspool.tile([128, PAIR], F32, tag="ps",
                                     name="ps_final")
                    for j in range(p2):
                        off = c0 + j * S + u * TILE
                        nc.tensor.matmul(ps[:, 0:TILE], w2[j][:],
                                         bufB[:, off: off + TILE],
                                         start=(j == 0), stop=(j == p2 - 1),
                                         skip_group_check=True)
                    # both unit copies on Act (DVE pays a long post-op DRAIN
                    # after its pair-14 half; Act is free here); stores on
                    # separate queues so the two ~0.6us issues overlap.
                    dst = outsb[:, c0 + u * TILE: c0 + (u + 1) * TILE]
                    nc.scalar.copy(dst, ps[:, 0:TILE])
                    q = nc.sync if u == 0 else nc.scalar
                    sdma(q, out_d[:, c0 + u * TILE: c0 + (u + 1) * TILE], dst)

            LAG = 2
            for p in range(NP_):
                cons = f1_consumer_split if p == NP_ - 1 else f1_consumer
                emit_pair(xbuf, w1, p, cons, bank_major=(p == 0))
                if p >= LAG:
                    emit_pair(bufB, w2, p - LAG,
                              lambda q, ps: f2_consumer(q, ps, "v"))
            emit_pair(bufB, w2, NP_ - 2,
                      lambda q, ps: f2_consumer(q, ps, "split"))
            emit_f2_final_units(NP_ - 1)

    nc.compile()
    return nc


def _get_program(p1, p2):
    key = (p1, p2)
    if key not in _PROGRAM_CACHE:
        _PROGRAM_CACHE[key] = _build_program(p1, p2)
    return _PROGRAM_CACHE[key]


# ---------------------------------------------------------------- host entry
def _shard_inputs(x):
    """x [B,T,C] fp32 -> list of per-core Xm [128, T] fp16 arrays.

    Xm[l, c*S + s] = seq[core*S + s][c*L + l].
    """
    xs = np.ascontiguousarray(np.transpose(np.asarray(x, dtype=np.float32),
                                           (0, 2, 1))).reshape(B * C, T)
    xs = xs.astype(np.float16)
    shards = []
    for core in range(NCORES):
        seqs = xs[core * S: (core + 1) * S]
        Xm = np.ascontiguousarray(
            seqs.reshape(S, N, L).transpose(2, 1, 0)).reshape(L, N * S)
        shards.append(Xm)
    return shards


def _unshard_output(outs):
    """list of per-core [128, T] fp16 device outputs -> [B, T, C] fp32."""
    ys = np.empty((B * C, T), dtype=np.float32)
    for core in range(NCORES):
        O = np.asarray(outs[core]).astype(np.float32)
        ys[core * S: (core + 1) * S] = (
            O.reshape(L, N, S).transpose(2, 1, 0).reshape(S, T))
    return np.ascontiguousarray(ys.reshape(B, C, T).transpose(0, 2, 1))


def kernel(x, bp_sos, lp_sos, _trace=False, **_ignored):
    from concourse.bass_utils import run_bass_kernel_spmd

    w1, p1 = _fir_weights(np.asarray(bp_sos))
    w2, p2 = _fir_weights(np.asarray(lp_sos))
    consts = {
        "w1": np.ascontiguousarray(np.concatenate(w1, axis=1)),
        "w2": np.ascontiguousarray(np.concatenate(w2, axis=1)),
    }
    shards = _shard_inputs(x)
    nc = _get_program(p1, p2)
    in_maps = [dict(consts, x=shards[core]) for core in range(NCORES)]
    res = run_bass_kernel_spmd(nc, in_maps, list(range(NCORES)), trace=_trace)
    out = _unshard_output([res.results[core]["out"] for core in range(NCORES)])
    if _trace:
        return out, res
    return out


if __name__ == "__main__":
    rng = np.random.default_rng(0)
    x = rng.standard_normal((B, T, C), dtype=np.float32)
    sh = _shard_inputs(x)
    rt = _unshard_output(sh)
    print("roundtrip max err (fp16 quant only):",
          np.abs(rt - x).max())
